# revision 13
# baseline (speedup 1.0000x reference)
"""Causal single-head attention (B=4, S=4096, D=768, fp32) on 8 Trainium2 NeuronCores.

Sharding: 2 cores per batch. The 16 query blocks (256 rows each) of a batch are
split between the pair so causal work balances (snake pairing); every core runs
the SAME compiled program — all per-core differences (which query rows, causal
masks) are shipped as data.

On-device algorithm (flash-attention style, fp32r matmuls = full PE speed with
~tf32 accuracy):
  phase 1: xT via PE transpose; project K^T and V(+ones col) to DRAM scratch,
           Q^T (own queries only) resident in SBUF.
  phase 2: stream K^T/V back in blocks of 4 k-chunks (kc-block OUTER loop, so
           each k block is read once); for each query slot still active, compute
           S^T = K Q^T in PSUM, exp on ScalarE (scale folded), mask via shipped
           per-core mask tiles on the final block, accumulate U = W @ [V|1] in
           PSUM, flush-add into an SBUF accumulator.
  phase 3: out = U[:, :768] * (1 / U[:, 768]) per row, DMA out.
"""

import numpy as np

import concourse.bacc as bacc
import concourse.mybir as mybir
import concourse.tile as tile
from concourse.bass_utils import run_bass_kernel_spmd
from concourse.masks import make_identity

B, S, D = 4, 4096, 768
P = 128
DC = D // P            # 6 feature chunks
NKC = S // P           # 32 key chunks
QB = 256               # query block width
NSLOT = 8              # query blocks per core (8 * 256 = 2048 queries)
SQ = NSLOT * QB        # 2048
KB = 4                 # key chunks per streamed block
NKB = NKC // KB        # 8
VW = 772               # V row width: 768 + ones col + pad
SM_SCALE = float(1.0 / np.sqrt(D))
N_CORES = 8

# Snake pairing of the 16 global query blocks between the two cores of a batch:
# slot j holds global block 2j or 2j+1; both cores run ceil-padded trip counts.
ASSIGN = [
    [(2 * j + 1) if j % 2 == 0 else (2 * j) for j in range(NSLOT)],  # role 0
    [(2 * j) if j % 2 == 0 else (2 * j + 1) for j in range(NSLOT)],  # role 1
]

ACT_F32R = True  # ScalarE may write float32r (rounding) — flip if compile rejects

_CACHE = {}


def _round_copy(nc, dst, src):
    if ACT_F32R:
        nc.scalar.copy(dst, src)
    else:
        nc.vector.tensor_copy(dst, src)


def _build_module():
    dt = mybir.dt
    f32, f32r = dt.float32, dt.float32r
    nc = bacc.Bacc("TRN2", target_bir_lowering=False, debug=False,
                   num_devices=N_CORES)

    xkv_d = nc.dram_tensor("xkv", [S // 2, D], f32, kind="ExternalInput").ap()
    xq_d = nc.dram_tensor("xq", [SQ, D], f32, kind="ExternalInput").ap()
    wq_d = nc.dram_tensor("Wq", [D, D], f32, kind="ExternalInput").ap()
    wk_d = nc.dram_tensor("Wk", [D, D], f32, kind="ExternalInput").ap()
    wv_d = nc.dram_tensor("Wv", [D, D], f32, kind="ExternalInput").ap()
    mask_d = nc.dram_tensor("masks", [NSLOT, KB, P, QB], f32r,
                            kind="ExternalInput").ap()
    out_d = nc.dram_tensor("out", [SQ, D], f32, kind="ExternalOutput").ap()

    xkv_r = xkv_d.rearrange("(a p) d -> p a d", p=P)  # [128, 16, 768]
    xq_r = xq_d.rearrange("(a p) d -> p a d", p=P)    # [128, 16, 768]
    out_r = out_d.rearrange("(a p) d -> p a d", p=P)  # [128, 16, 768]

    Exp = mybir.ActivationFunctionType.Exp

    with tile.TileContext(nc) as tc:
        with tc.tile_pool(name="singles", bufs=1) as singles, \
             tc.tile_pool(name="dram", bufs=1, space="DRAM") as dram:
            ident = singles.tile([P, P], f32)
            make_identity(nc, ident)
            qt = singles.tile([P, DC, SQ], f32r)      # Q^T resident
            # per-512-k-block KT|V contribution (this core's half) and the
            # pair-AllGathered result [rank, ...]
            CW = DC * 4 * P + KB * VW                 # 6160 floats
            ktv_in = [dram.tile([P, CW], f32r, tag=f"ktvi{j}", name=f"ktvi{j}")
                      for j in range(4)]
            ktv_out = [dram.tile([2, P, CW], f32r, tag=f"ktvo{j}", name=f"ktvo{j}")
                       for j in range(4)]

            # ---------------- phase 1: projections ----------------
            with tc.tile_pool(name="wload", bufs=1) as wload, \
                 tc.tile_pool(name="weights", bufs=1) as weights, \
                 tc.tile_pool(name="p1", bufs=2) as p1, \
                 tc.tile_pool(name="stage", bufs=3) as stage, \
                 tc.tile_pool(name="pst", bufs=2, space="PSUM") as pst, \
                 tc.tile_pool(name="psp", bufs=2, space="PSUM") as psp, \
                 tc.tile_pool(name="psv", bufs=2, space="PSUM") as psv:
                w_r = {}
                for name, wd in (("wq", wq_d), ("wk", wk_d), ("wv", wv_d)):
                    wf = wload.tile([P, DC, D], f32, tag="wf")
                    nc.sync.dma_start(out=wf, in_=wd.rearrange("(c p) e -> p c e", p=P))
                    wr = weights.tile([P, DC, D], f32r, tag="w_" + name)
                    nc.vector.tensor_copy(wr, wf)
                    w_r[name] = wr

                def transpose_block(src_r, a0):
                    xs = p1.tile([P, 4, D], f32, tag="xs")
                    nc.sync.dma_start(out=xs, in_=src_r[:, a0:a0 + 4, :])
                    xt = p1.tile([P, DC, 4 * P], f32r, tag="xt")
                    for sc in range(4):
                        for c in range(DC):
                            pt = pst.tile([P, P], f32, tag="pt")
                            nc.tensor.transpose(pt, xs[:, sc, c * P:(c + 1) * P], ident)
                            nc.vector.tensor_copy(xt[:, c, sc * P:(sc + 1) * P], pt)
                    return xt

                def proj_q_block(blk):
                    xt = transpose_block(xq_r, blk * 4)
                    w = w_r["wq"]
                    for ec in range(DC):
                        pp = psp.tile([P, 4 * P], f32, tag="pp")
                        for c in range(DC):
                            nc.tensor.matmul(pp, w[:, c, ec * P:(ec + 1) * P],
                                             xt[:, c, :],
                                             start=(c == 0), stop=(c == DC - 1))
                        _round_copy(nc, qt[:, ec, blk * 4 * P:(blk + 1) * 4 * P], pp)

                def proj_kv_block(jj):
                    # my local k-block jj (global 512-block 2*jj + role)
                    xt = transpose_block(xkv_r, jj * 4)
                    w = w_r["wk"]
                    for ec in range(DC):
                        pp = psp.tile([P, 4 * P], f32, tag="pp")
                        for c in range(DC):
                            nc.tensor.matmul(pp, w[:, c, ec * P:(ec + 1) * P],
                                             xt[:, c, :],
                                             start=(c == 0), stop=(c == DC - 1))
                        stk = stage.tile([P, 4 * P], f32r, tag="stk")
                        _round_copy(nc, stk, pp)
                        nc.sync.dma_start(
                            out=ktv_in[jj][:, ec * 4 * P:(ec + 1) * 4 * P], in_=stk)
                    wv = w_r["wv"]
                    voff = DC * 4 * P
                    for sc in range(4):
                        pv = psv.tile([P, VW], f32, tag="pv")
                        for c in range(DC):
                            lhs = xt[:, c, sc * P:(sc + 1) * P]
                            nc.tensor.matmul(pv[:, 0:512], lhs, wv[:, c, 0:512],
                                             start=(c == 0), stop=(c == DC - 1))
                            nc.tensor.matmul(pv[:, 512:768], lhs, wv[:, c, 512:768],
                                             start=(c == 0), stop=(c == DC - 1))
                        sv = stage.tile([P, VW], f32r, tag="sv")
                        _round_copy(nc, sv[:, 0:768], pv[:, 0:768])
                        nc.vector.memset(sv[:, 768:769].bitcast(mybir.dt.float32), 1.0)
                        nc.vector.memset(sv[:, 769:VW].bitcast(mybir.dt.float32), 0.0)
                        nc.sync.dma_start(
                            out=ktv_in[jj][:, voff + sc * VW:voff + (sc + 1) * VW],
                            in_=sv)
                    nc.gpsimd.collective_compute(
                        "AllGather", mybir.AluOpType.bypass,
                        replica_groups=[[0, 1], [2, 3], [4, 5], [6, 7]],
                        ins=[ktv_in[jj][:]], outs=[ktv_out[jj][:]],
                    )

                for blk in range(SQ // (4 * P)):
                    proj_q_block(blk)
                for jj in range(4):
                    proj_kv_block(jj)

            # ---------------- phase 2: attention ----------------
            with tc.tile_pool(name="uaccp", bufs=1) as uaccp, \
                 tc.tile_pool(name="ring", bufs=2) as ring, \
                 tc.tile_pool(name="wtp", bufs=3) as wtp, \
                 tc.tile_pool(name="mring", bufs=2) as mring, \
                 tc.tile_pool(name="fin", bufs=2) as fin, \
                 tc.tile_pool(name="pss", bufs=3, space="PSUM") as pss, \
                 tc.tile_pool(name="psu", bufs=1, space="PSUM") as psu:
                uacc = uaccp.tile([P, 2 * NSLOT, VW], f32)
                VOFF = DC * 4 * P
                for kb in range(NKB):
                    rt = ring.tile([P, CW], f32r, tag="rt")
                    nc.sync.dma_start(out=rt, in_=ktv_out[kb // 2][kb % 2])
                    for j in range(kb, NSLOT):
                        mb = None
                        if kb == j:
                            mb = mring.tile([P, KB, QB], f32r, tag="mb")
                            nc.sync.dma_start(
                                out=mb, in_=mask_d[j].rearrange("t p f -> p t f"))
                        pu0 = psu.tile([P, VW], f32, tag="pu0")
                        pu1 = psu.tile([P, VW], f32, tag="pu1")
                        pus = (pu0, pu1)
                        for t in range(KB):
                            ps = pss.tile([P, QB], f32, tag="ps")
                            for c in range(DC):
                                nc.tensor.matmul(
                                    ps, rt[:, c * 4 * P + t * P:c * 4 * P + (t + 1) * P],
                                    qt[:, c, j * QB:(j + 1) * QB],
                                    start=(c == 0), stop=(c == DC - 1))
                            wt = wtp.tile([P, QB], f32r, tag="wt")
                            if ACT_F32R:
                                nc.scalar.activation(wt, ps, Exp, scale=SM_SCALE)
                            else:
                                wtf = wtp.tile([P, QB], f32, tag="wtf")
                                nc.scalar.activation(wtf, ps, Exp, scale=SM_SCALE)
                                nc.vector.tensor_copy(wt, wtf)
                            if mb is not None:
                                nc.vector.tensor_mul(wt, wt, mb[:, t, :])
                            vrow = VOFF + t * VW
                            for qc in range(2):
                                lhs = wt[:, qc * P:(qc + 1) * P]
                                nc.tensor.matmul(pus[qc][:, 0:512], lhs,
                                                 rt[:, vrow:vrow + 512],
                                                 start=(t == 0), stop=(t == KB - 1))
                                nc.tensor.matmul(pus[qc][:, 512:VW], lhs,
                                                 rt[:, vrow + 512:vrow + VW],
                                                 start=(t == 0), stop=(t == KB - 1))
                        for qc in range(2):
                            dst = uacc[:, 2 * j + qc, 0:769]
                            if kb == 0:
                                nc.scalar.copy(dst, pus[qc][:, 0:769])
                            else:
                                nc.vector.tensor_add(dst, dst, pus[qc][:, 0:769])

                # ---------------- phase 3: normalize + store ----------------
                for j in range(NSLOT):
                    for qc in range(2):
                        sl = 2 * j + qc
                        zr = fin.tile([P, 1], f32, tag="zr")
                        nc.vector.reciprocal(zr, uacc[:, sl, 768:769])
                        ob = fin.tile([P, D], f32, tag="ob")
                        nc.scalar.mul(ob, uacc[:, sl, 0:768], zr)
                        nc.sync.dma_start(out=out_r[:, sl, :], in_=ob)

    nc.compile()
    return nc


def _get_module():
    if "nc" not in _CACHE:
        _CACHE["nc"] = _build_module()
    return _CACHE["nc"]


def _build_masks(chunks):
    m = np.zeros((NSLOT, KB, P, QB), np.float32)
    prow = np.arange(P)[:, None]
    fcol = np.arange(QB)[None, :]
    for j, g in enumerate(chunks):
        for t in range(KB):
            kc = KB * j + t
            m[j, t] = (prow <= fcol + (g * QB - kc * P)).astype(np.float32)
    return m


def _make_in_maps(inputs):
    x = np.asarray(inputs["x"], np.float32)
    Wq = np.ascontiguousarray(np.asarray(inputs["Wq"], np.float32))
    Wk = np.ascontiguousarray(np.asarray(inputs["Wk"], np.float32))
    Wv = np.ascontiguousarray(np.asarray(inputs["Wv"], np.float32))
    in_maps = []
    for c in range(N_CORES):
        b, r = c // 2, c % 2
        chunks = ASSIGN[r]
        xb = x[b]
        xq = np.ascontiguousarray(
            xb.reshape(S // QB, QB, D)[chunks].reshape(SQ, D))
        xkv = np.ascontiguousarray(
            xb.reshape(8, S // 8, D)[r::2].reshape(S // 2, D))
        in_maps.append({
            "xkv": xkv, "xq": xq, "Wq": Wq, "Wk": Wk, "Wv": Wv,
            "masks": _build_masks(chunks),
        })
    return in_maps


def _run(inputs, trace=False, trace_kwargs=None):
    nc = _get_module()
    in_maps = _make_in_maps(inputs)

    kw = {}
    if trace:
        kw["trace"] = True
        kw["trace_cores"] = (trace_kwargs or {}).pop("trace_cores", None) \
            or list(range(N_CORES))
        if trace_kwargs:
            kw["trace_kwargs"] = trace_kwargs
    res = run_bass_kernel_spmd(nc, in_maps, core_ids=list(range(N_CORES)), **kw)

    out = np.empty((B, S, D), np.float32)
    for c in range(N_CORES):
        b, r = c // 2, c % 2
        o = res.results[c]["out"].reshape(NSLOT, QB, D)
        for j, g in enumerate(ASSIGN[r]):
            out[b, g * QB:(g + 1) * QB] = o[j]
    return out, res


def kernel(**inputs) -> np.ndarray:
    out, _ = _run(inputs, trace=False)
    return out


# revision 19
# speedup vs baseline: 1.1809x; 1.1809x over previous
"""Causal single-head attention (B=4, S=4096, D=768, fp32) on 8 Trainium2 NeuronCores.

Sharding: 2 cores per batch. The 16 query blocks (256 rows each) of a batch are
split between the pair so causal work balances (snake pairing); every core runs
the SAME compiled program — all per-core differences (which query rows, causal
masks) are shipped as data.

On-device algorithm (flash-attention style, fp32r matmuls = full PE speed with
~tf32 accuracy):
  phase 1: xT via PE transpose; project K^T and V(+ones col) to DRAM scratch,
           Q^T (own queries only) resident in SBUF.
  phase 2: stream K^T/V back in blocks of 4 k-chunks (kc-block OUTER loop, so
           each k block is read once); for each query slot still active, compute
           S^T = K Q^T in PSUM, exp on ScalarE (scale folded), mask via shipped
           per-core mask tiles on the final block, accumulate U = W @ [V|1] in
           PSUM, flush-add into an SBUF accumulator.
  phase 3: out = U[:, :768] * (1 / U[:, 768]) per row, DMA out.
"""

import numpy as np

import concourse.bacc as bacc
import concourse.mybir as mybir
import concourse.tile as tile
from concourse.bass_utils import run_bass_kernel_spmd
from concourse.masks import make_identity

B, S, D = 4, 4096, 768
P = 128
DC = D // P            # 6 feature chunks
NKC = S // P           # 32 key chunks
QB = 256               # query block width
NSLOT = 8              # query blocks per core (8 * 256 = 2048 queries)
SQ = NSLOT * QB        # 2048
KB = 4                 # key chunks per streamed block
NKB = NKC // KB        # 8
VW = 772               # V row width: 768 + ones col + pad
SM_SCALE = float(1.0 / np.sqrt(D))
N_CORES = 8

# Snake pairing of the 16 global query blocks between the two cores of a batch:
# slot j holds global block 2j or 2j+1; both cores run ceil-padded trip counts.
ASSIGN = [
    [(2 * j + 1) if j % 2 == 0 else (2 * j) for j in range(NSLOT)],  # role 0
    [(2 * j) if j % 2 == 0 else (2 * j + 1) for j in range(NSLOT)],  # role 1
]

ACT_F32R = True  # ScalarE may write float32r (rounding) — flip if compile rejects

_CACHE = {}


def _round_copy(nc, dst, src):
    if ACT_F32R:
        nc.scalar.copy(dst, src)
    else:
        nc.vector.tensor_copy(dst, src)


def _build_module():
    dt = mybir.dt
    f32, f32r = dt.float32, dt.float32r
    nc = bacc.Bacc("TRN2", target_bir_lowering=False, debug=False,
                   num_devices=N_CORES)

    x_d = nc.dram_tensor("x", [S, D], f32, kind="ExternalInput").ap()
    xq_d = nc.dram_tensor("xq", [SQ, D], f32, kind="ExternalInput").ap()
    wq_d = nc.dram_tensor("Wq", [D, D], f32, kind="ExternalInput").ap()
    wk_d = nc.dram_tensor("Wk", [D, D], f32, kind="ExternalInput").ap()
    wv_d = nc.dram_tensor("Wv", [D, D], f32, kind="ExternalInput").ap()
    mask_d = nc.dram_tensor("masks", [NSLOT, KB, P, QB], f32r,
                            kind="ExternalInput").ap()
    out_d = nc.dram_tensor("out", [SQ, D], f32, kind="ExternalOutput").ap()

    x_r = x_d.rearrange("(a p) d -> p a d", p=P)      # [128, 32, 768]
    xq_r = xq_d.rearrange("(a p) d -> p a d", p=P)    # [128, 16, 768]
    out_r = out_d.rearrange("(a p) d -> p a d", p=P)  # [128, 16, 768]

    Exp = mybir.ActivationFunctionType.Exp

    with tile.TileContext(nc) as tc:
        with tc.tile_pool(name="singles", bufs=1) as singles, \
             tc.tile_pool(name="dram", bufs=1, space="DRAM") as dram:
            ident = singles.tile([P, P], f32)
            make_identity(nc, ident)
            qt = singles.tile([P, DC, SQ], f32r)      # Q^T resident
            # per-512-k-block KT|V scratch; block 0 stays in SBUF (kb=0 reads
            # it directly, no DRAM round-trip)
            CW = DC * 4 * P + KB * VW                 # 6160 floats
            ktv = [dram.tile([P, CW], f32r, tag=f"ktv{j}", name=f"ktv{j}")
                   for j in range(1, NKB)]
            ktv0 = singles.tile([P, CW], f32r)

            # ---------------- phase 1: projections ----------------
            with tc.tile_pool(name="wload", bufs=1) as wload, \
                 tc.tile_pool(name="weights", bufs=1) as weights, \
                 tc.tile_pool(name="p1", bufs=2) as p1, \
                 tc.tile_pool(name="stage", bufs=3) as stage, \
                 tc.tile_pool(name="pst", bufs=2, space="PSUM") as pst, \
                 tc.tile_pool(name="psp", bufs=2, space="PSUM") as psp, \
                 tc.tile_pool(name="psv", bufs=2, space="PSUM") as psv:
                w_r = {}
                for name, wd in (("wq", wq_d), ("wk", wk_d), ("wv", wv_d)):
                    wf = wload.tile([P, DC, D], f32, tag="wf")
                    nc.sync.dma_start(out=wf, in_=wd.rearrange("(c p) e -> p c e", p=P))
                    wr = weights.tile([P, DC, D], f32r, tag="w_" + name)
                    nc.vector.tensor_copy(wr, wf)
                    w_r[name] = wr

                def transpose_block(src_r, a0):
                    xs = p1.tile([P, 4, D], f32, tag="xs")
                    nc.sync.dma_start(out=xs, in_=src_r[:, a0:a0 + 4, :])
                    xt = p1.tile([P, DC, 4 * P], f32r, tag="xt")
                    for sc in range(4):
                        for c in range(DC):
                            pt = pst.tile([P, P], f32, tag="pt")
                            nc.tensor.transpose(pt, xs[:, sc, c * P:(c + 1) * P], ident)
                            nc.vector.tensor_copy(xt[:, c, sc * P:(sc + 1) * P], pt)
                    return xt

                def proj_q_block(blk):
                    xt = transpose_block(xq_r, blk * 4)
                    w = w_r["wq"]
                    for ec in range(DC):
                        pp = psp.tile([P, 4 * P], f32, tag="pp")
                        for c in range(DC):
                            nc.tensor.matmul(pp, w[:, c, ec * P:(ec + 1) * P],
                                             xt[:, c, :],
                                             start=(c == 0), stop=(c == DC - 1))
                        _round_copy(nc, qt[:, ec, blk * 4 * P:(blk + 1) * 4 * P], pp)

                def proj_kv_block(jj):
                    # global 512-row k-block jj; block 0 lands in SBUF (ktv0)
                    xt = transpose_block(x_r, jj * 4)
                    w = w_r["wk"]
                    for ec in range(DC):
                        pp = psp.tile([P, 4 * P], f32, tag="pp")
                        for c in range(DC):
                            nc.tensor.matmul(pp, w[:, c, ec * P:(ec + 1) * P],
                                             xt[:, c, :],
                                             start=(c == 0), stop=(c == DC - 1))
                        if jj == 0:
                            _round_copy(nc, ktv0[:, ec * 4 * P:(ec + 1) * 4 * P], pp)
                        else:
                            stk = stage.tile([P, 4 * P], f32r, tag="stk")
                            _round_copy(nc, stk, pp)
                            nc.sync.dma_start(
                                out=ktv[jj - 1][:, ec * 4 * P:(ec + 1) * 4 * P],
                                in_=stk)
                    wv = w_r["wv"]
                    voff = DC * 4 * P
                    for sc in range(4):
                        pv = psv.tile([P, VW], f32, tag="pv")
                        for c in range(DC):
                            lhs = xt[:, c, sc * P:(sc + 1) * P]
                            nc.tensor.matmul(pv[:, 0:512], lhs, wv[:, c, 0:512],
                                             start=(c == 0), stop=(c == DC - 1))
                            nc.tensor.matmul(pv[:, 512:768], lhs, wv[:, c, 512:768],
                                             start=(c == 0), stop=(c == DC - 1))
                        if jj == 0:
                            dst = ktv0[:, voff + sc * VW:voff + (sc + 1) * VW]
                            _round_copy(nc, dst[:, 0:768], pv[:, 0:768])
                            nc.vector.memset(dst[:, 768:769].bitcast(mybir.dt.float32), 1.0)
                            nc.vector.memset(dst[:, 769:VW].bitcast(mybir.dt.float32), 0.0)
                        else:
                            sv = stage.tile([P, VW], f32r, tag="sv")
                            _round_copy(nc, sv[:, 0:768], pv[:, 0:768])
                            nc.vector.memset(sv[:, 768:769].bitcast(mybir.dt.float32), 1.0)
                            nc.vector.memset(sv[:, 769:VW].bitcast(mybir.dt.float32), 0.0)
                            nc.sync.dma_start(
                                out=ktv[jj - 1][:, voff + sc * VW:voff + (sc + 1) * VW],
                                in_=sv)

                proj_kv_block(0)
                for blk in range(SQ // (4 * P)):
                    proj_q_block(blk)
                for jj in range(1, NKB):
                    proj_kv_block(jj)

            # ---------------- phase 2: attention ----------------
            with tc.tile_pool(name="uaccp", bufs=1) as uaccp, \
                 tc.tile_pool(name="ring", bufs=2) as ring, \
                 tc.tile_pool(name="wtp", bufs=3) as wtp, \
                 tc.tile_pool(name="mring", bufs=2) as mring, \
                 tc.tile_pool(name="fin", bufs=2) as fin, \
                 tc.tile_pool(name="pss", bufs=3, space="PSUM") as pss, \
                 tc.tile_pool(name="psu", bufs=1, space="PSUM") as psu:
                uacc = uaccp.tile([P, 2 * NSLOT, VW], f32)
                VOFF = DC * 4 * P
                for kb in range(NKB):
                    if kb == 0:
                        rt = ktv0
                    else:
                        rt = ring.tile([P, CW], f32r, tag="rt")
                        nc.sync.dma_start(out=rt, in_=ktv[kb - 1])
                    for j in range(kb, NSLOT):
                        mb = None
                        if kb == j:
                            mb = mring.tile([P, KB, QB], f32r, tag="mb")
                            nc.sync.dma_start(
                                out=mb, in_=mask_d[j].rearrange("t p f -> p t f"))
                        pu0 = psu.tile([P, VW], f32, tag="pu0")
                        pu1 = psu.tile([P, VW], f32, tag="pu1")
                        pus = (pu0, pu1)
                        for t in range(KB):
                            ps = pss.tile([P, QB], f32, tag="ps")
                            for c in range(DC):
                                nc.tensor.matmul(
                                    ps, rt[:, c * 4 * P + t * P:c * 4 * P + (t + 1) * P],
                                    qt[:, c, j * QB:(j + 1) * QB],
                                    start=(c == 0), stop=(c == DC - 1))
                            wt = wtp.tile([P, QB], f32r, tag="wt")
                            if ACT_F32R:
                                nc.scalar.activation(wt, ps, Exp, scale=SM_SCALE)
                            else:
                                wtf = wtp.tile([P, QB], f32, tag="wtf")
                                nc.scalar.activation(wtf, ps, Exp, scale=SM_SCALE)
                                nc.vector.tensor_copy(wt, wtf)
                            if mb is not None:
                                nc.vector.tensor_mul(wt, wt, mb[:, t, :])
                            vrow = VOFF + t * VW
                            for qc in range(2):
                                lhs = wt[:, qc * P:(qc + 1) * P]
                                nc.tensor.matmul(pus[qc][:, 0:512], lhs,
                                                 rt[:, vrow:vrow + 512],
                                                 start=(t == 0), stop=(t == KB - 1))
                                nc.tensor.matmul(pus[qc][:, 512:VW], lhs,
                                                 rt[:, vrow + 512:vrow + VW],
                                                 start=(t == 0), stop=(t == KB - 1))
                        for qc in range(2):
                            dst = uacc[:, 2 * j + qc, 0:769]
                            if kb == 0:
                                nc.scalar.copy(dst, pus[qc][:, 0:769])
                            else:
                                nc.vector.tensor_add(dst, dst, pus[qc][:, 0:769])

                # ---------------- phase 3: normalize + store ----------------
                for j in range(NSLOT):
                    for qc in range(2):
                        sl = 2 * j + qc
                        zr = fin.tile([P, 1], f32, tag="zr")
                        nc.vector.reciprocal(zr, uacc[:, sl, 768:769])
                        ob = fin.tile([P, D], f32, tag="ob")
                        nc.scalar.mul(ob, uacc[:, sl, 0:768], zr)
                        nc.sync.dma_start(out=out_r[:, sl, :], in_=ob)

    nc.compile()
    return nc


def _get_module():
    if "nc" not in _CACHE:
        _CACHE["nc"] = _build_module()
    return _CACHE["nc"]


def _build_masks(chunks):
    m = np.zeros((NSLOT, KB, P, QB), np.float32)
    prow = np.arange(P)[:, None]
    fcol = np.arange(QB)[None, :]
    for j, g in enumerate(chunks):
        for t in range(KB):
            kc = KB * j + t
            m[j, t] = (prow <= fcol + (g * QB - kc * P)).astype(np.float32)
    return m


def _make_in_maps(inputs):
    x = np.asarray(inputs["x"], np.float32)
    Wq = np.ascontiguousarray(np.asarray(inputs["Wq"], np.float32))
    Wk = np.ascontiguousarray(np.asarray(inputs["Wk"], np.float32))
    Wv = np.ascontiguousarray(np.asarray(inputs["Wv"], np.float32))
    in_maps = []
    for c in range(N_CORES):
        b, r = c // 2, c % 2
        chunks = ASSIGN[r]
        xb = x[b]
        xq = np.ascontiguousarray(
            xb.reshape(S // QB, QB, D)[chunks].reshape(SQ, D))
        in_maps.append({
            "x": np.ascontiguousarray(xb), "xq": xq,
            "Wq": Wq, "Wk": Wk, "Wv": Wv,
            "masks": _build_masks(chunks),
        })
    return in_maps


def _run(inputs, trace=False, trace_kwargs=None):
    nc = _get_module()
    in_maps = _make_in_maps(inputs)

    kw = {}
    if trace:
        kw["trace"] = True
        kw["trace_cores"] = (trace_kwargs or {}).pop("trace_cores", None) \
            or list(range(N_CORES))
        if trace_kwargs:
            kw["trace_kwargs"] = trace_kwargs
    res = run_bass_kernel_spmd(nc, in_maps, core_ids=list(range(N_CORES)), **kw)

    out = np.empty((B, S, D), np.float32)
    for c in range(N_CORES):
        b, r = c // 2, c % 2
        o = res.results[c]["out"].reshape(NSLOT, QB, D)
        for j, g in enumerate(ASSIGN[r]):
            out[b, g * QB:(g + 1) * QB] = o[j]
    return out, res


def kernel(**inputs) -> np.ndarray:
    out, _ = _run(inputs, trace=False)
    return out


# revision 20
# speedup vs baseline: 1.2277x; 1.0396x over previous
"""Causal single-head attention (B=4, S=4096, D=768, fp32) on 8 Trainium2 NeuronCores.

Sharding: 2 cores per batch, split over the KEY dimension by interleaved
128-row chunks (core role r owns key chunks kc with kc % 2 == r). Each core
computes, for every query block, the partial flash-attention accumulator
U = sum_k w * [V | 1] over its own keys; the host merges the two partials per
batch (add, then divide by the gathered Z column). The interleaved split makes
causal work identical on both cores, so one compiled program serves all 8
cores — every per-core difference (which rows are "mine", the diagonal mask)
is shipped as data:
  - x arrives row-permuted: this core's key chunks first, the peer's second.
  - Q^T is projected for ALL queries in that permuted column order (to DRAM
    scratch); the per-query-block gather happens in the phase-2 ring load.
  - the single diagonal-mask tile is [tril | ones] for role 0, [tril | zeros]
    for role 1 (query columns are stored [own-chunk | peer-chunk] per block;
    the host swaps the halves back for role 1).

All matmuls run in float32r (~tf32 accuracy, full PE speed). exp on ScalarE
with the softmax scale folded in; no max-subtraction (scores are O(1)).
"""

import numpy as np

import concourse.bacc as bacc
import concourse.mybir as mybir
import concourse.tile as tile
from concourse.bass_utils import run_bass_kernel_spmd
from concourse.masks import make_identity

B, S, D = 4, 4096, 768
P = 128
DC = D // P            # 6 feature chunks
QB = 256               # query block width
NSLOT = S // QB        # 16 query blocks (all queries, every core)
NT = 16                # key chunks owned per core (16 * 128 = 2048 keys)
VW = 772               # V row width: 768 + ones col + pad (f32r MM needs N%4==0)
SM_SCALE = float(1.0 / np.sqrt(D))
N_CORES = 8

ACT_F32R = True  # ScalarE may write float32r (rounding)

_CACHE = {}


def _round_copy(nc, dst, src):
    if ACT_F32R:
        nc.scalar.copy(dst, src)
    else:
        nc.vector.tensor_copy(dst, src)


def _build_module():
    dt = mybir.dt
    f32, f32r = dt.float32, dt.float32r
    nc = bacc.Bacc("TRN2", target_bir_lowering=False, debug=False,
                   num_devices=N_CORES)

    x_d = nc.dram_tensor("x", [S, D], f32, kind="ExternalInput").ap()
    wq_d = nc.dram_tensor("Wq", [D, D], f32, kind="ExternalInput").ap()
    wk_d = nc.dram_tensor("Wk", [D, D], f32, kind="ExternalInput").ap()
    wv_d = nc.dram_tensor("Wv", [D, D], f32, kind="ExternalInput").ap()
    dmask_d = nc.dram_tensor("dmask", [P, QB], f32r, kind="ExternalInput").ap()
    out_d = nc.dram_tensor("out", [S, VW], f32, kind="ExternalOutput").ap()

    x_r = x_d.rearrange("(a p) d -> p a d", p=P)      # [128, 32, 768]
    out_r = out_d.rearrange("(a p) e -> p a e", p=P)  # [128, 32, 772]

    Exp = mybir.ActivationFunctionType.Exp

    with tile.TileContext(nc) as tc:
        with tc.tile_pool(name="singles", bufs=1) as singles, \
             tc.tile_pool(name="dram", bufs=1, space="DRAM") as dram:
            ident = singles.tile([P, P], f32)
            make_identity(nc, ident)
            dmask = singles.tile([P, QB], f32r)
            nc.sync.dma_start(out=dmask, in_=dmask_d)
            kt = singles.tile([P, DC, NT * P], f32r)   # K^T (own keys) resident
            vv = singles.tile([P, NT, VW], f32r)       # V|1 (own keys) resident
            # Q^T for all queries, x_perm column order: [p, c, 2 regions, 2048]
            qt_t = dram.tile([P, DC, S], f32r, name="qt_scratch")

            # ---------------- phase 1: transposes + projections ----------------
            with tc.tile_pool(name="wload", bufs=1) as wload, \
                 tc.tile_pool(name="weights", bufs=1) as weights, \
                 tc.tile_pool(name="p1", bufs=1) as p1, \
                 tc.tile_pool(name="stage", bufs=3) as stage, \
                 tc.tile_pool(name="pst", bufs=2, space="PSUM") as pst, \
                 tc.tile_pool(name="psp", bufs=2, space="PSUM") as psp, \
                 tc.tile_pool(name="psv", bufs=2, space="PSUM") as psv:
                w_r = {}
                for name, wd in (("wq", wq_d), ("wk", wk_d), ("wv", wv_d)):
                    wf = wload.tile([P, DC, D], f32, tag="wf")
                    nc.sync.dma_start(out=wf, in_=wd.rearrange("(c p) e -> p c e", p=P))
                    wr = weights.tile([P, DC, D], f32r, tag="w_" + name)
                    nc.vector.tensor_copy(wr, wf)
                    w_r[name] = wr

                def proj_block(b):
                    # x_perm rows [b*512, (b+1)*512); b < 4 -> my key chunks
                    xs = p1.tile([P, 4, D], f32, tag="xs")
                    nc.sync.dma_start(out=xs, in_=x_r[:, b * 4:(b + 1) * 4, :])
                    xt = p1.tile([P, DC, 4 * P], f32r, tag="xt")
                    for sc in range(4):
                        for c in range(DC):
                            pt = pst.tile([P, P], f32, tag="pt")
                            nc.tensor.transpose(pt, xs[:, sc, c * P:(c + 1) * P], ident)
                            nc.vector.tensor_copy(xt[:, c, sc * P:(sc + 1) * P], pt)
                    # Q^T for these 512 permuted columns -> DRAM scratch
                    w = w_r["wq"]
                    for ec in range(DC):
                        pp = psp.tile([P, 4 * P], f32, tag="pp")
                        for c in range(DC):
                            nc.tensor.matmul(pp, w[:, c, ec * P:(ec + 1) * P],
                                             xt[:, c, :],
                                             start=(c == 0), stop=(c == DC - 1))
                        stq = stage.tile([P, 4 * P], f32r, tag="stq")
                        _round_copy(nc, stq, pp)
                        nc.sync.dma_start(
                            out=qt_t[:, ec, b * 4 * P:(b + 1) * 4 * P], in_=stq)
                    if b >= 4:
                        return
                    # K^T and V for my 4 key chunks of this block
                    w = w_r["wk"]
                    for ec in range(DC):
                        pp = psp.tile([P, 4 * P], f32, tag="pp")
                        for c in range(DC):
                            nc.tensor.matmul(pp, w[:, c, ec * P:(ec + 1) * P],
                                             xt[:, c, :],
                                             start=(c == 0), stop=(c == DC - 1))
                        nc.vector.tensor_copy(
                            kt[:, ec, b * 4 * P:(b + 1) * 4 * P], pp)
                    wv = w_r["wv"]
                    for sc in range(4):
                        pv = psv.tile([P, VW], f32, tag="pv")
                        for c in range(DC):
                            lhs = xt[:, c, sc * P:(sc + 1) * P]
                            nc.tensor.matmul(pv[:, 0:512], lhs, wv[:, c, 0:512],
                                             start=(c == 0), stop=(c == DC - 1))
                            nc.tensor.matmul(pv[:, 512:768], lhs, wv[:, c, 512:768],
                                             start=(c == 0), stop=(c == DC - 1))
                        dst = vv[:, b * 4 + sc, :]
                        _round_copy(nc, dst[:, 0:768], pv[:, 0:768])
                        nc.vector.memset(dst[:, 768:769].bitcast(f32), 1.0)
                        nc.vector.memset(dst[:, 769:VW].bitcast(f32), 0.0)

                for b in (0, 4, 1, 5, 2, 6, 3, 7):
                    proj_block(b)

            # ---------------- phase 2: attention (partial, own keys) ----------
            qt_g = qt_t[:].rearrange("p c (h s) -> p c h s", h=2)  # regions
            with tc.tile_pool(name="qring", bufs=2) as qring, \
                 tc.tile_pool(name="wtp", bufs=3) as wtp, \
                 tc.tile_pool(name="fin", bufs=2) as fin, \
                 tc.tile_pool(name="pss", bufs=3, space="PSUM") as pss, \
                 tc.tile_pool(name="psu", bufs=1, space="PSUM") as psu:
                for g in range(NSLOT):
                    # Q^T columns of slot g: [own chunk g | peer chunk g]
                    qs = qring.tile([P, DC, 2, P], f32r, tag="qs")
                    nc.sync.dma_start(out=qs,
                                      in_=qt_g[:, :, :, g * P:(g + 1) * P])
                    pu0 = psu.tile([P, VW], f32, tag="pu0")
                    pu1 = psu.tile([P, VW], f32, tag="pu1")
                    pus = (pu0, pu1)
                    for t in range(g + 1):
                        ps = pss.tile([P, QB], f32, tag="ps")
                        for c in range(DC):
                            nc.tensor.matmul(ps, kt[:, c, t * P:(t + 1) * P],
                                             qs[:, c, :, :],
                                             start=(c == 0), stop=(c == DC - 1))
                        wt = wtp.tile([P, QB], f32r, tag="wt")
                        if ACT_F32R:
                            nc.scalar.activation(wt, ps, Exp, scale=SM_SCALE)
                        else:
                            wtf = wtp.tile([P, QB], f32, tag="wtf")
                            nc.scalar.activation(wtf, ps, Exp, scale=SM_SCALE)
                            nc.vector.tensor_copy(wt, wtf)
                        if t == g:
                            nc.vector.tensor_mul(wt, wt, dmask)
                        for qc in range(2):
                            lhs = wt[:, qc * P:(qc + 1) * P]
                            nc.tensor.matmul(pus[qc][:, 0:512], lhs,
                                             vv[:, t, 0:512],
                                             start=(t == 0), stop=(t == g))
                            nc.tensor.matmul(pus[qc][:, 512:VW], lhs,
                                             vv[:, t, 512:VW],
                                             start=(t == 0), stop=(t == g))
                    ob = fin.tile([P, 2, VW], f32, tag="ob")
                    for qc in range(2):
                        nc.scalar.copy(ob[:, qc, :], pus[qc])
                    nc.sync.dma_start(out=out_r[:, 2 * g:2 * g + 2, :], in_=ob)

    nc.compile()
    return nc


def _get_module():
    if "nc" not in _CACHE:
        _CACHE["nc"] = _build_module()
    return _CACHE["nc"]


def _make_in_maps(inputs):
    x = np.asarray(inputs["x"], np.float32)
    Wq = np.ascontiguousarray(np.asarray(inputs["Wq"], np.float32))
    Wk = np.ascontiguousarray(np.asarray(inputs["Wk"], np.float32))
    Wv = np.ascontiguousarray(np.asarray(inputs["Wv"], np.float32))
    tril = (np.arange(P)[:, None] <= np.arange(P)[None, :]).astype(np.float32)
    in_maps = []
    for c in range(N_CORES):
        b, r = c // 2, c % 2
        xb = x[b].reshape(S // P, P, D)
        x_perm = np.ascontiguousarray(
            np.concatenate([xb[r::2], xb[1 - r::2]]).reshape(S, D))
        half = np.ones((P, P), np.float32) if r == 0 else np.zeros((P, P), np.float32)
        dmask = np.ascontiguousarray(np.concatenate([tril, half], axis=1))
        in_maps.append({
            "x": x_perm, "Wq": Wq, "Wk": Wk, "Wv": Wv, "dmask": dmask,
        })
    return in_maps


def _run(inputs, trace=False, trace_kwargs=None):
    nc = _get_module()
    in_maps = _make_in_maps(inputs)

    kw = {}
    if trace:
        kw["trace"] = True
        kw["trace_cores"] = (trace_kwargs or {}).pop("trace_cores", None) \
            or list(range(N_CORES))
        if trace_kwargs:
            kw["trace_kwargs"] = trace_kwargs
    res = run_bass_kernel_spmd(nc, in_maps, core_ids=list(range(N_CORES)), **kw)

    out = np.empty((B, S, D), np.float32)
    for b in range(B):
        u0 = res.results[2 * b]["out"]
        u1 = res.results[2 * b + 1]["out"]
        # role-1 stores each query block as [odd chunk | even chunk]; swap back
        u1 = u1.reshape(NSLOT, 2, P, VW)[:, ::-1].reshape(S, VW)
        u = u0 + u1
        out[b] = u[:, 0:D] / u[:, D:D + 1]
    return out, res


def kernel(**inputs) -> np.ndarray:
    out, _ = _run(inputs, trace=False)
    return out


# revision 22
# speedup vs baseline: 1.2356x; 1.0064x over previous
"""Causal single-head attention (B=4, S=4096, D=768, fp32) on 8 Trainium2 NeuronCores.

Sharding: 2 cores per batch, split over the KEY dimension by interleaved
128-row chunks (core role r owns key chunks kc with kc % 2 == r). Each core
computes, for every query block, the partial flash-attention accumulator
U = sum_k w * [V | 1] over its own keys; the host merges the two partials per
batch (add, then divide by the gathered Z column). The interleaved split makes
causal work identical on both cores, so one compiled program serves all 8
cores — every per-core difference (which rows are "mine", the diagonal mask)
is shipped as data:
  - x arrives row-permuted: this core's key chunks first, the peer's second.
  - Q^T is projected for ALL queries in that permuted column order (to DRAM
    scratch); the per-query-block gather happens in the phase-2 ring load.
  - the single diagonal-mask tile is [tril | ones] for role 0, [tril | zeros]
    for role 1 (query columns are stored [own-chunk | peer-chunk] per block;
    the host swaps the halves back for role 1).

All matmuls run in float32r (~tf32 accuracy, full PE speed). exp on ScalarE
with the softmax scale folded in; no max-subtraction (scores are O(1)).
"""

import numpy as np

import concourse.bacc as bacc
import concourse.mybir as mybir
import concourse.tile as tile
from concourse.bass_utils import run_bass_kernel_spmd
from concourse.masks import make_identity

B, S, D = 4, 4096, 768
P = 128
DC = D // P            # 6 feature chunks
QB = 256               # query block width
NSLOT = S // QB        # 16 query blocks (all queries, every core)
NT = 16                # key chunks owned per core (16 * 128 = 2048 keys)
VW = 772               # V row width: 768 + ones col + pad (f32r MM needs N%4==0)
SM_SCALE = float(1.0 / np.sqrt(D))
N_CORES = 8

ACT_F32R = True  # ScalarE may write float32r (rounding)

_CACHE = {}


def _round_copy(nc, dst, src):
    if ACT_F32R:
        nc.scalar.copy(dst, src)
    else:
        nc.vector.tensor_copy(dst, src)


def _build_module():
    dt = mybir.dt
    f32, f32r = dt.float32, dt.float32r
    nc = bacc.Bacc("TRN2", target_bir_lowering=False, debug=False,
                   num_devices=N_CORES)

    x_d = nc.dram_tensor("x", [S, D], f32, kind="ExternalInput").ap()
    wq_d = nc.dram_tensor("Wq", [D, D], f32, kind="ExternalInput").ap()
    wk_d = nc.dram_tensor("Wk", [D, D], f32, kind="ExternalInput").ap()
    wv_d = nc.dram_tensor("Wv", [D, D], f32, kind="ExternalInput").ap()
    dmask_d = nc.dram_tensor("dmask", [P, QB], f32r, kind="ExternalInput").ap()
    out_d = nc.dram_tensor("out", [S, VW], f32, kind="ExternalOutput").ap()

    x_r = x_d.rearrange("(a p) d -> p a d", p=P)      # [128, 32, 768]
    out_r = out_d.rearrange("(a p) e -> p a e", p=P)  # [128, 32, 772]

    Exp = mybir.ActivationFunctionType.Exp

    with tile.TileContext(nc) as tc:
        with tc.tile_pool(name="singles", bufs=1) as singles, \
             tc.tile_pool(name="dram", bufs=1, space="DRAM") as dram:
            ident = singles.tile([P, P], f32)
            make_identity(nc, ident)
            dmask = singles.tile([P, QB], f32r)
            nc.sync.dma_start(out=dmask, in_=dmask_d)
            kt = singles.tile([P, DC, NT * P], f32r)   # K^T (own keys) resident
            vv = singles.tile([P, NT, VW], f32r)       # V|1 (own keys) resident
            # Q^T for all queries, x_perm column order: [p, c, 2 regions, 2048]
            qt_t = dram.tile([P, DC, S], f32r, name="qt_scratch")

            # ---------------- phase 1: transposes + projections ----------------
            with tc.tile_pool(name="wload", bufs=1) as wload, \
                 tc.tile_pool(name="weights", bufs=1) as weights, \
                 tc.tile_pool(name="p1", bufs=1) as p1, \
                 tc.tile_pool(name="stage", bufs=3) as stage, \
                 tc.tile_pool(name="pst", bufs=2, space="PSUM") as pst, \
                 tc.tile_pool(name="psp", bufs=2, space="PSUM") as psp, \
                 tc.tile_pool(name="psv", bufs=2, space="PSUM") as psv:
                w_r = {}
                for name, wd in (("wq", wq_d), ("wk", wk_d), ("wv", wv_d)):
                    wf = wload.tile([P, DC, D], f32, tag="wf")
                    nc.sync.dma_start(out=wf, in_=wd.rearrange("(c p) e -> p c e", p=P))
                    wr = weights.tile([P, DC, D], f32r, tag="w_" + name)
                    nc.vector.tensor_copy(wr, wf)
                    w_r[name] = wr

                def proj_block(b):
                    # x_perm rows [b*512, (b+1)*512); b < 4 -> my key chunks
                    xs = p1.tile([P, 4, D], f32, tag="xs")
                    nc.sync.dma_start(out=xs, in_=x_r[:, b * 4:(b + 1) * 4, :])
                    xt = p1.tile([P, DC, 4 * P], f32r, tag="xt")
                    for sc in range(4):
                        for c in range(DC):
                            pt = pst.tile([P, P], f32, tag="pt")
                            nc.tensor.transpose(pt, xs[:, sc, c * P:(c + 1) * P], ident)
                            nc.vector.tensor_copy(xt[:, c, sc * P:(sc + 1) * P], pt)
                    # Q^T for these 512 permuted columns -> DRAM scratch
                    w = w_r["wq"]
                    for ec in range(DC):
                        pp = psp.tile([P, 4 * P], f32, tag="pp")
                        nc.tensor.matmul(pp, w[:, 0, ec * P:(ec + 1) * P],
                                         xt[:, 0, :], start=True, stop=False)
                        for c in range(1, DC):
                            for h in range(2):
                                nc.tensor.matmul(
                                    pp[:, h * 256:(h + 1) * 256],
                                    w[:, c, ec * P:(ec + 1) * P],
                                    xt[:, c, h * 256:(h + 1) * 256],
                                    start=False, stop=(c == DC - 1))
                        stq = stage.tile([P, 4 * P], f32r, tag="stq")
                        _round_copy(nc, stq, pp)
                        nc.sync.dma_start(
                            out=qt_t[:, ec, b * 4 * P:(b + 1) * 4 * P], in_=stq)
                    if b >= 4:
                        return
                    # K^T and V for my 4 key chunks of this block
                    w = w_r["wk"]
                    for ec in range(DC):
                        pp = psp.tile([P, 4 * P], f32, tag="pp")
                        nc.tensor.matmul(pp, w[:, 0, ec * P:(ec + 1) * P],
                                         xt[:, 0, :], start=True, stop=False)
                        for c in range(1, DC):
                            for h in range(2):
                                nc.tensor.matmul(
                                    pp[:, h * 256:(h + 1) * 256],
                                    w[:, c, ec * P:(ec + 1) * P],
                                    xt[:, c, h * 256:(h + 1) * 256],
                                    start=False, stop=(c == DC - 1))
                        nc.vector.tensor_copy(
                            kt[:, ec, b * 4 * P:(b + 1) * 4 * P], pp)
                    wv = w_r["wv"]
                    for sc in range(4):
                        pv = psv.tile([P, VW], f32, tag="pv")
                        lhs = xt[:, 0, sc * P:(sc + 1) * P]
                        nc.tensor.matmul(pv[:, 0:512], lhs, wv[:, 0, 0:512],
                                         start=True, stop=False)
                        nc.tensor.matmul(pv[:, 512:768], lhs, wv[:, 0, 512:768],
                                         start=True, stop=False)
                        for c in range(1, DC):
                            lhs = xt[:, c, sc * P:(sc + 1) * P]
                            for h in range(3):
                                nc.tensor.matmul(
                                    pv[:, h * 256:(h + 1) * 256], lhs,
                                    wv[:, c, h * 256:(h + 1) * 256],
                                    start=False, stop=(c == DC - 1))
                        dst = vv[:, b * 4 + sc, :]
                        _round_copy(nc, dst[:, 0:768], pv[:, 0:768])
                        nc.vector.memset(dst[:, 768:769].bitcast(f32), 1.0)
                        nc.vector.memset(dst[:, 769:VW].bitcast(f32), 0.0)

                for b in (0, 4, 1, 5, 2, 6, 3, 7):
                    proj_block(b)

            # ---------------- phase 2: attention (partial, own keys) ----------
            qt_g = qt_t[:].rearrange("p c (h s) -> p c h s", h=2)  # regions
            with tc.tile_pool(name="qring", bufs=2) as qring, \
                 tc.tile_pool(name="wtp", bufs=3) as wtp, \
                 tc.tile_pool(name="fin", bufs=2) as fin, \
                 tc.tile_pool(name="pss", bufs=3, space="PSUM") as pss, \
                 tc.tile_pool(name="psu", bufs=1, space="PSUM") as psu:
                for g in range(NSLOT):
                    # Q^T columns of slot g: [own chunk g | peer chunk g]
                    qs = qring.tile([P, DC, 2, P], f32r, tag="qs")
                    nc.sync.dma_start(out=qs,
                                      in_=qt_g[:, :, :, g * P:(g + 1) * P])
                    pu0 = psu.tile([P, VW], f32, tag="pu0")
                    pu1 = psu.tile([P, VW], f32, tag="pu1")
                    pus = (pu0, pu1)
                    for t in range(g + 1):
                        ps = pss.tile([P, QB], f32, tag="ps")
                        for c in range(DC):
                            nc.tensor.matmul(ps, kt[:, c, t * P:(t + 1) * P],
                                             qs[:, c, :, :],
                                             start=(c == 0), stop=(c == DC - 1))
                        wt = wtp.tile([P, QB], f32r, tag="wt")
                        if ACT_F32R:
                            nc.scalar.activation(wt, ps, Exp, scale=SM_SCALE)
                        else:
                            wtf = wtp.tile([P, QB], f32, tag="wtf")
                            nc.scalar.activation(wtf, ps, Exp, scale=SM_SCALE)
                            nc.vector.tensor_copy(wt, wtf)
                        if t == g:
                            nc.vector.tensor_mul(wt, wt, dmask)
                        for qc in range(2):
                            lhs = wt[:, qc * P:(qc + 1) * P]
                            if t == 0:
                                nc.tensor.matmul(pus[qc][:, 0:512], lhs,
                                                 vv[:, t, 0:512],
                                                 start=True, stop=(t == g))
                            else:
                                nc.tensor.matmul(pus[qc][:, 0:256], lhs,
                                                 vv[:, t, 0:256],
                                                 start=False, stop=(t == g))
                                nc.tensor.matmul(pus[qc][:, 256:512], lhs,
                                                 vv[:, t, 256:512],
                                                 start=False, stop=(t == g))
                            nc.tensor.matmul(pus[qc][:, 512:VW], lhs,
                                             vv[:, t, 512:VW],
                                             start=(t == 0), stop=(t == g))
                    ob = fin.tile([P, 2, VW], f32, tag="ob")
                    for qc in range(2):
                        nc.scalar.copy(ob[:, qc, :], pus[qc])
                    nc.sync.dma_start(out=out_r[:, 2 * g:2 * g + 2, :], in_=ob)

    nc.compile()
    return nc


def _get_module():
    if "nc" not in _CACHE:
        _CACHE["nc"] = _build_module()
    return _CACHE["nc"]


def _make_in_maps(inputs):
    x = np.asarray(inputs["x"], np.float32)
    Wq = np.ascontiguousarray(np.asarray(inputs["Wq"], np.float32))
    Wk = np.ascontiguousarray(np.asarray(inputs["Wk"], np.float32))
    Wv = np.ascontiguousarray(np.asarray(inputs["Wv"], np.float32))
    tril = (np.arange(P)[:, None] <= np.arange(P)[None, :]).astype(np.float32)
    in_maps = []
    for c in range(N_CORES):
        b, r = c // 2, c % 2
        xb = x[b].reshape(S // P, P, D)
        x_perm = np.ascontiguousarray(
            np.concatenate([xb[r::2], xb[1 - r::2]]).reshape(S, D))
        half = np.ones((P, P), np.float32) if r == 0 else np.zeros((P, P), np.float32)
        dmask = np.ascontiguousarray(np.concatenate([tril, half], axis=1))
        in_maps.append({
            "x": x_perm, "Wq": Wq, "Wk": Wk, "Wv": Wv, "dmask": dmask,
        })
    return in_maps


def _run(inputs, trace=False, trace_kwargs=None):
    nc = _get_module()
    in_maps = _make_in_maps(inputs)

    kw = {}
    if trace:
        kw["trace"] = True
        kw["trace_cores"] = (trace_kwargs or {}).pop("trace_cores", None) \
            or list(range(N_CORES))
        if trace_kwargs:
            kw["trace_kwargs"] = trace_kwargs
    res = run_bass_kernel_spmd(nc, in_maps, core_ids=list(range(N_CORES)), **kw)

    out = np.empty((B, S, D), np.float32)
    for b in range(B):
        u0 = res.results[2 * b]["out"]
        u1 = res.results[2 * b + 1]["out"]
        # role-1 stores each query block as [odd chunk | even chunk]; swap back
        u1 = u1.reshape(NSLOT, 2, P, VW)[:, ::-1].reshape(S, VW)
        u = u0 + u1
        out[b] = u[:, 0:D] / u[:, D:D + 1]
    return out, res


def kernel(**inputs) -> np.ndarray:
    out, _ = _run(inputs, trace=False)
    return out


# revision 23
# speedup vs baseline: 1.2834x; 1.0387x over previous
"""Causal single-head attention (B=4, S=4096, D=768, fp32) on 8 Trainium2 NeuronCores.

Sharding: 2 cores per batch, split over the KEY dimension by interleaved
128-row chunks (core role r owns key chunks kc with kc % 2 == r). Each core
computes, for every query block, the partial flash-attention accumulator
U = sum_k w * [V | 1] over its own keys; the host merges the two partials per
batch (add, then divide by the gathered Z column). The interleaved split makes
causal work identical on both cores, so one compiled program serves all 8
cores — every per-core difference (which rows are "mine", the diagonal mask)
is shipped as data:
  - x arrives row-permuted: this core's key chunks first, the peer's second.
  - Q^T is projected for ALL queries in that permuted column order (to DRAM
    scratch); the per-query-block gather happens in the phase-2 ring load.
  - the single diagonal-mask tile is [tril | ones] for role 0, [tril | zeros]
    for role 1 (query columns are stored [own-chunk | peer-chunk] per block;
    the host swaps the halves back for role 1).

All matmuls run in float32r (~tf32 accuracy, full PE speed). exp on ScalarE
with the softmax scale folded in; no max-subtraction (scores are O(1)).
"""

import numpy as np

import concourse.bacc as bacc
import concourse.mybir as mybir
import concourse.tile as tile
from concourse.bass_utils import run_bass_kernel_spmd
from concourse.masks import make_identity

B, S, D = 4, 4096, 768
P = 128
DC = D // P            # 6 feature chunks
QB = 256               # query block width
NSLOT = S // QB        # 16 query blocks (all queries, every core)
NT = 16                # key chunks owned per core (16 * 128 = 2048 keys)
VW = 772               # V row width: 768 + ones col + pad (f32r MM needs N%4==0)
SM_SCALE = float(1.0 / np.sqrt(D))
N_CORES = 8

ACT_F32R = True  # ScalarE may write float32r (rounding)

_CACHE = {}


def _round_copy(nc, dst, src):
    if ACT_F32R:
        nc.scalar.copy(dst, src)
    else:
        nc.vector.tensor_copy(dst, src)


def _build_module():
    dt = mybir.dt
    f32, f32r = dt.float32, dt.float32r
    nc = bacc.Bacc("TRN2", target_bir_lowering=False, debug=False,
                   num_devices=N_CORES)

    x_d = nc.dram_tensor("x", [S, D], f32, kind="ExternalInput").ap()
    wq_d = nc.dram_tensor("Wq", [D, D], f32, kind="ExternalInput").ap()
    wk_d = nc.dram_tensor("Wk", [D, D], f32, kind="ExternalInput").ap()
    wv_d = nc.dram_tensor("Wv", [D, D], f32, kind="ExternalInput").ap()
    dmask_d = nc.dram_tensor("dmask", [P, QB], f32r, kind="ExternalInput").ap()
    out_d = nc.dram_tensor("out", [S, VW], f32, kind="ExternalOutput").ap()

    x_r = x_d.rearrange("(a p) d -> p a d", p=P)      # [128, 32, 768]
    out_r = out_d.rearrange("(a p) e -> p a e", p=P)  # [128, 32, 772]

    Exp = mybir.ActivationFunctionType.Exp

    with tile.TileContext(nc) as tc:
        with tc.tile_pool(name="singles", bufs=1) as singles, \
             tc.tile_pool(name="dram", bufs=1, space="DRAM") as dram:
            ident = singles.tile([P, P], f32)
            make_identity(nc, ident)
            dmask = singles.tile([P, QB], f32r)
            nc.sync.dma_start(out=dmask, in_=dmask_d)
            kt = singles.tile([P, DC, NT * P], f32r)   # K^T (own keys) resident
            vv = singles.tile([P, NT, VW], f32r)       # V|1 (own keys) resident
            # Q^T for all queries, x_perm column order: [p, c, 2 regions, 2048]
            qt_t = dram.tile([P, DC, S], f32r, name="qt_scratch")

            # ---------------- phase 1: transposes + projections ----------------
            with tc.tile_pool(name="wload", bufs=1) as wload, \
                 tc.tile_pool(name="weights", bufs=1) as weights, \
                 tc.tile_pool(name="p1", bufs=1) as p1, \
                 tc.tile_pool(name="stage", bufs=3) as stage, \
                 tc.tile_pool(name="pst", bufs=2, space="PSUM") as pst, \
                 tc.tile_pool(name="psp", bufs=2, space="PSUM") as psp, \
                 tc.tile_pool(name="psv", bufs=2, space="PSUM") as psv:
                # first x block loads go out before the bulky weight DMAs so
                # the PE can start transposing at ~5us instead of ~20us
                xs0 = p1.tile([P, 4, D], f32, tag="xs")
                nc.sync.dma_start(out=xs0, in_=x_r[:, 0:4, :])
                w_r = {}
                for name, wd in (("wq", wq_d), ("wk", wk_d), ("wv", wv_d)):
                    wf = wload.tile([P, DC, D], f32, tag="wf")
                    nc.sync.dma_start(out=wf, in_=wd.rearrange("(c p) e -> p c e", p=P))
                    wr = weights.tile([P, DC, D], f32r, tag="w_" + name)
                    nc.vector.tensor_copy(wr, wf)
                    w_r[name] = wr

                def proj_block(b):
                    # x_perm rows [b*512, (b+1)*512); b < 4 -> my key chunks
                    if b == 0:
                        xs = xs0
                    else:
                        xs = p1.tile([P, 4, D], f32, tag="xs")
                        nc.sync.dma_start(out=xs, in_=x_r[:, b * 4:(b + 1) * 4, :])
                    xt = p1.tile([P, DC, 4 * P], f32r, tag="xt")
                    for sc in range(4):
                        for c in range(DC):
                            pt = pst.tile([P, P], f32, tag="pt")
                            nc.tensor.transpose(pt, xs[:, sc, c * P:(c + 1) * P], ident)
                            nc.vector.tensor_copy(xt[:, c, sc * P:(sc + 1) * P], pt)
                    # Q^T for these 512 permuted columns -> DRAM scratch
                    w = w_r["wq"]
                    for ec in range(DC):
                        pp = psp.tile([P, 4 * P], f32, tag="pp")
                        nc.tensor.matmul(pp, w[:, 0, ec * P:(ec + 1) * P],
                                         xt[:, 0, :], start=True, stop=False)
                        for c in range(1, DC):
                            for h in range(2):
                                nc.tensor.matmul(
                                    pp[:, h * 256:(h + 1) * 256],
                                    w[:, c, ec * P:(ec + 1) * P],
                                    xt[:, c, h * 256:(h + 1) * 256],
                                    start=False, stop=(c == DC - 1))
                        stq = stage.tile([P, 4 * P], f32r, tag="stq")
                        _round_copy(nc, stq, pp)
                        nc.sync.dma_start(
                            out=qt_t[:, ec, b * 4 * P:(b + 1) * 4 * P], in_=stq)
                    if b >= 4:
                        return
                    # K^T and V for my 4 key chunks of this block
                    w = w_r["wk"]
                    for ec in range(DC):
                        pp = psp.tile([P, 4 * P], f32, tag="pp")
                        nc.tensor.matmul(pp, w[:, 0, ec * P:(ec + 1) * P],
                                         xt[:, 0, :], start=True, stop=False)
                        for c in range(1, DC):
                            for h in range(2):
                                nc.tensor.matmul(
                                    pp[:, h * 256:(h + 1) * 256],
                                    w[:, c, ec * P:(ec + 1) * P],
                                    xt[:, c, h * 256:(h + 1) * 256],
                                    start=False, stop=(c == DC - 1))
                        nc.vector.tensor_copy(
                            kt[:, ec, b * 4 * P:(b + 1) * 4 * P], pp)
                    wv = w_r["wv"]
                    for sc in range(4):
                        pv = psv.tile([P, VW], f32, tag="pv")
                        lhs = xt[:, 0, sc * P:(sc + 1) * P]
                        nc.tensor.matmul(pv[:, 0:512], lhs, wv[:, 0, 0:512],
                                         start=True, stop=False)
                        nc.tensor.matmul(pv[:, 512:768], lhs, wv[:, 0, 512:768],
                                         start=True, stop=False)
                        for c in range(1, DC):
                            lhs = xt[:, c, sc * P:(sc + 1) * P]
                            for h in range(3):
                                nc.tensor.matmul(
                                    pv[:, h * 256:(h + 1) * 256], lhs,
                                    wv[:, c, h * 256:(h + 1) * 256],
                                    start=False, stop=(c == DC - 1))
                        dst = vv[:, b * 4 + sc, :]
                        _round_copy(nc, dst[:, 0:768], pv[:, 0:768])
                        nc.vector.memset(dst[:, 768:769].bitcast(f32), 1.0)
                        nc.vector.memset(dst[:, 769:VW].bitcast(f32), 0.0)

                for b in (0, 4, 1, 5, 2, 6, 3, 7):
                    proj_block(b)

            # ---------------- phase 2: attention (partial, own keys) ----------
            qt_g = qt_t[:].rearrange("p c (h s) -> p c h s", h=2)  # regions
            with tc.tile_pool(name="qring", bufs=2) as qring, \
                 tc.tile_pool(name="wtp", bufs=3) as wtp, \
                 tc.tile_pool(name="fin", bufs=2) as fin, \
                 tc.tile_pool(name="pss", bufs=3, space="PSUM") as pss, \
                 tc.tile_pool(name="psu", bufs=1, space="PSUM") as psu:
                for g in range(NSLOT):
                    # Q^T columns of slot g: [own chunk g | peer chunk g]
                    qs = qring.tile([P, DC, 2, P], f32r, tag="qs")
                    nc.sync.dma_start(out=qs,
                                      in_=qt_g[:, :, :, g * P:(g + 1) * P])
                    pu0 = psu.tile([P, VW], f32, tag="pu0")
                    pu1 = psu.tile([P, VW], f32, tag="pu1")
                    pus = (pu0, pu1)
                    for t in range(g + 1):
                        ps = pss.tile([P, QB], f32, tag="ps")
                        for c in range(DC):
                            nc.tensor.matmul(ps, kt[:, c, t * P:(t + 1) * P],
                                             qs[:, c, :, :],
                                             start=(c == 0), stop=(c == DC - 1))
                        wt = wtp.tile([P, QB], f32r, tag="wt")
                        if ACT_F32R:
                            nc.scalar.activation(wt, ps, Exp, scale=SM_SCALE)
                        else:
                            wtf = wtp.tile([P, QB], f32, tag="wtf")
                            nc.scalar.activation(wtf, ps, Exp, scale=SM_SCALE)
                            nc.vector.tensor_copy(wt, wtf)
                        if t == g:
                            nc.vector.tensor_mul(wt, wt, dmask)
                        for qc in range(2):
                            lhs = wt[:, qc * P:(qc + 1) * P]
                            if t == 0:
                                nc.tensor.matmul(pus[qc][:, 0:512], lhs,
                                                 vv[:, t, 0:512],
                                                 start=True, stop=(t == g))
                            else:
                                nc.tensor.matmul(pus[qc][:, 0:256], lhs,
                                                 vv[:, t, 0:256],
                                                 start=False, stop=(t == g))
                                nc.tensor.matmul(pus[qc][:, 256:512], lhs,
                                                 vv[:, t, 256:512],
                                                 start=False, stop=(t == g))
                            nc.tensor.matmul(pus[qc][:, 512:VW], lhs,
                                             vv[:, t, 512:VW],
                                             start=(t == 0), stop=(t == g))
                    ob = fin.tile([P, 2, VW], f32, tag="ob")
                    for qc in range(2):
                        nc.scalar.copy(ob[:, qc, :], pus[qc])
                    nc.sync.dma_start(out=out_r[:, 2 * g:2 * g + 2, :], in_=ob)

    nc.compile()
    return nc


def _get_module():
    if "nc" not in _CACHE:
        _CACHE["nc"] = _build_module()
    return _CACHE["nc"]


def _make_in_maps(inputs):
    x = np.asarray(inputs["x"], np.float32)
    Wq = np.ascontiguousarray(np.asarray(inputs["Wq"], np.float32))
    Wk = np.ascontiguousarray(np.asarray(inputs["Wk"], np.float32))
    Wv = np.ascontiguousarray(np.asarray(inputs["Wv"], np.float32))
    tril = (np.arange(P)[:, None] <= np.arange(P)[None, :]).astype(np.float32)
    in_maps = []
    for c in range(N_CORES):
        b, r = c // 2, c % 2
        xb = x[b].reshape(S // P, P, D)
        x_perm = np.ascontiguousarray(
            np.concatenate([xb[r::2], xb[1 - r::2]]).reshape(S, D))
        half = np.ones((P, P), np.float32) if r == 0 else np.zeros((P, P), np.float32)
        dmask = np.ascontiguousarray(np.concatenate([tril, half], axis=1))
        in_maps.append({
            "x": x_perm, "Wq": Wq, "Wk": Wk, "Wv": Wv, "dmask": dmask,
        })
    return in_maps


def _run(inputs, trace=False, trace_kwargs=None):
    nc = _get_module()
    in_maps = _make_in_maps(inputs)

    kw = {}
    if trace:
        kw["trace"] = True
        kw["trace_cores"] = (trace_kwargs or {}).pop("trace_cores", None) \
            or list(range(N_CORES))
        if trace_kwargs:
            kw["trace_kwargs"] = trace_kwargs
    res = run_bass_kernel_spmd(nc, in_maps, core_ids=list(range(N_CORES)), **kw)

    out = np.empty((B, S, D), np.float32)
    for b in range(B):
        u0 = res.results[2 * b]["out"]
        u1 = res.results[2 * b + 1]["out"]
        # role-1 stores each query block as [odd chunk | even chunk]; swap back
        u1 = u1.reshape(NSLOT, 2, P, VW)[:, ::-1].reshape(S, VW)
        u = u0 + u1
        out[b] = u[:, 0:D] / u[:, D:D + 1]
    return out, res


def kernel(**inputs) -> np.ndarray:
    out, _ = _run(inputs, trace=False)
    return out
